# revision 6
# baseline (speedup 1.0000x reference)
# Trainium2 Bass kernel for nn_BigramLanguageModel (4-layer dense transformer + vocab head).
#
# Sharding (8 NeuronCores, no collectives):
#   - Trunk (4 transformer layers): data-parallel over batch. Core c computes the
#     full trunk for batch element c//2 over all T=1024 tokens (each batch is
#     computed by the 2 cores of a pair; redundant but exchange-free).
#   - Vocab head: tensor-parallel over V. Core c projects all 1024 tokens of its
#     batch against its half of Wout (V/2 = 16000 columns, parity c%2), writes
#     float16 logits and fused exp-row-sum partials for the loss.
#   - Host: embedding gather, LN gain/bias folding into weights, logits assembly
#     (f16 -> f32), log-sum-exp combine across vocab halves, final loss.
#
# All matmuls run in float32r (full PE speed, ~1.5e-4 rel error vs ~6e-3 for bf16).
import numpy as np

B, T, E, H, L, V = 4, 1024, 256, 4, 4, 32000
HD = E // H              # 64
EPS = 1e-5
NCORES = 8
VH = V // 2              # 16000 vocab columns per core
FF = 4 * E               # 1024
TT = T // 128            # 8 token tiles
VCH = 512                # vocab chunk
NVC = (VH + VCH - 1) // VCH  # 32 chunks (31x512 + 1x128)

_cache = {}


def _build_program(with_bv, with_bproj, with_b2, with_bout, debug=False):
    import concourse.bass as bass
    import concourse.tile as tile
    from concourse import bacc, mybir

    f32 = mybir.dt.float32
    f32r = mybir.dt.float32r
    f16 = mybir.dt.float16
    bf16 = mybir.dt.bfloat16
    AF = mybir.ActivationFunctionType

    nc = bacc.Bacc("TRN2", target_bir_lowering=False, debug=False, num_devices=NCORES)

    # ---- DRAM I/O ----
    x0_d = nc.dram_tensor("x0", [T, E], f32, kind="ExternalInput").ap()
    wq_d = nc.dram_tensor("wq", [L, E, E], f32r, kind="ExternalInput").ap()
    wk_d = nc.dram_tensor("wk", [L, E, E], f32r, kind="ExternalInput").ap()
    wv_d = nc.dram_tensor("wv", [L, E, E], f32r, kind="ExternalInput").ap()
    wp_d = nc.dram_tensor("wp", [L, E, E], f32r, kind="ExternalInput").ap()
    w1_d = nc.dram_tensor("w1", [L, E, FF], f32r, kind="ExternalInput").ap()
    w2_d = nc.dram_tensor("w2", [L, FF, E], f32r, kind="ExternalInput").ap()
    bq_d = nc.dram_tensor("bq", [L, E], f32, kind="ExternalInput").ap()
    bk_d = nc.dram_tensor("bk", [L, E], f32, kind="ExternalInput").ap()
    b1_d = nc.dram_tensor("b1", [L, FF], f32, kind="ExternalInput").ap()
    bv_d = nc.dram_tensor("bvx", [L, E], f32, kind="ExternalInput").ap() if with_bv else None
    bpj_d = nc.dram_tensor("bpj", [L, E], f32, kind="ExternalInput").ap() if with_bproj else None
    b2_d = nc.dram_tensor("b2x", [L, E], f32, kind="ExternalInput").ap() if with_b2 else None
    wo_d = nc.dram_tensor("wout", [E, VH], f32r, kind="ExternalInput").ap()
    bo_d = nc.dram_tensor("bout", [VH], f32r, kind="ExternalInput").ap() if with_bout else None
    tri_d = nc.dram_tensor("tri2", [128, 2, 128], f32r, kind="ExternalInput").ap()
    id_d = nc.dram_tensor("ident", [128, 128], f32, kind="ExternalInput").ap()
    z2_d = nc.dram_tensor("zeros2", [128, 2, VCH], f32r, kind="ExternalInput").ap()
    oa_d = nc.dram_tensor("ones_aug", [128, TT * H], f32r, kind="ExternalInput").ap()
    on_d = nc.dram_tensor("ones_row", [1, 128], f32r, kind="ExternalInput").ap() if with_bout else None
    logits_d = nc.dram_tensor("logits_h", [T, VH], f16, kind="ExternalOutput").ap()
    if debug:
        xdbg_d = nc.dram_tensor("xdbg", [L, T, E], f32, kind="ExternalOutput").ap()
        hT0_d = nc.dram_tensor("hT0", [2, 128, T], f32, kind="ExternalOutput").ap()
        qT0_d = nc.dram_tensor("qT0", [2, 128, T], f32, kind="ExternalOutput").ap()
        v0_d = nc.dram_tensor("v0", [128, TT, H, HD + 1], f32, kind="ExternalOutput").ap()
        oall0_d = nc.dram_tensor("oall0", [128, TT, E], f32, kind="ExternalOutput").ap()
        xmid_d = nc.dram_tensor("xmid", [L, T, E], f32, kind="ExternalOutput").ap()
    rowsum_d = nc.dram_tensor("rowsum", [T], f32, kind="ExternalOutput").ap()

    with tile.TileContext(nc) as tc:
        from contextlib import ExitStack
        est = ExitStack()
        with est:
            # ---- kernel-lifetime SBUF pools ----
            const = est.enter_context(tc.tile_pool(name="const", bufs=1))
            xpool = est.enter_context(tc.tile_pool(name="x", bufs=1))
            vapool = est.enter_context(tc.tile_pool(name="vaug", bufs=1))
            htpool = est.enter_context(tc.tile_pool(name="ht", bufs=2))
            qkpool = est.enter_context(tc.tile_pool(name="qk", bufs=1))
            otpool = est.enter_context(tc.tile_pool(name="ot", bufs=1))
            oapool = est.enter_context(tc.tile_pool(name="oall", bufs=1))
            wpool = est.enter_context(tc.tile_pool(name="w", bufs=2))
            hpool = est.enter_context(tc.tile_pool(name="h", bufs=3))
            spool = est.enter_context(tc.tile_pool(name="stats", bufs=4))
            expool = est.enter_context(tc.tile_pool(name="ex", bufs=3))
            aopool = est.enter_context(tc.tile_pool(name="oaT", bufs=2))
            f1pool = est.enter_context(tc.tile_pool(name="ff1", bufs=1))
            rpool = est.enter_context(tc.tile_pool(name="recip", bufs=8))
            wopool = est.enter_context(tc.tile_pool(name="wo", bufs=3))
            lstpool = est.enter_context(tc.tile_pool(name="lst", bufs=4))
            edpool = est.enter_context(tc.tile_pool(name="ed", bufs=2))
            rspool = est.enter_context(tc.tile_pool(name="rs", bufs=1))

            # ---- constants ----
            tri2 = const.tile([128, 2, 128], f32r)
            nc.sync.dma_start(out=tri2, in_=tri_d)
            ident = const.tile([128, 128], f32)
            nc.sync.dma_start(out=ident, in_=id_d)
            zeros2 = const.tile([128, 2, VCH], f32r)
            nc.sync.dma_start(out=zeros2, in_=z2_d)
            eps_sb = const.tile([128, 1], f32)
            nc.vector.memset(eps_sb[:], EPS)
            if with_bout:
                ones_row = const.tile([1, 128], f32r)
                nc.sync.dma_start(out=ones_row, in_=on_d)
                bo_sb = const.tile([1, VH], f32r)
                nc.sync.dma_start(out=bo_sb, in_=bo_d[None, :])

            # ---- residual stream + v_aug ----
            x = xpool.tile([128, TT, E], f32)
            nc.sync.dma_start(out=x, in_=x0_d.rearrange("(t p) e -> p t e", p=128))
            v_aug = vapool.tile([128, TT, H, HD + 1], f32r)
            nc.sync.dma_start(out=v_aug[:, :, :, HD], in_=oa_d.rearrange("p (t h) -> p t h", t=TT))

            def ln_block(x_ap, ps_ptr):
                """Standardize (x - mean) * rsqrt(var + eps) along free dim; gains/biases
                are folded into downstream weights on the host."""
                st6 = spool.tile([128, 6], f32, tag="st6")
                nc.vector.bn_stats(out=st6[:], in_=x_ap)
                mv = spool.tile([128, 2], f32, tag="mv")
                nc.vector.bn_aggr(out=mv[:], in_=st6[:])
                sd = spool.tile([128, 1], f32, tag="sd")
                nc.scalar.activation(out=sd[:], in_=mv[:, 1:2], func=AF.Sqrt,
                                     bias=eps_sb[:, 0:1], scale=1.0)
                rstd = spool.tile([128, 1], f32, tag="rstd")
                nc.vector.reciprocal(out=rstd[:], in_=sd[:])
                mr = spool.tile([128, 1], f32, tag="mr")
                nc.vector.tensor_mul(mr[:], mv[:, 0:1], rstd[:])
                nmr = spool.tile([128, 1], f32, tag="nmr")
                nc.vector.tensor_scalar_mul(nmr[:], mr[:], -1.0)
                h = hpool.tile([128, E], f32, tag="h")
                nc.scalar.activation(out=h[:], in_=x_ap, func=AF.Identity,
                                     bias=nmr[:, 0:1], scale=rstd[:, 0:1])
                return h

            def transpose_into(dst3, src_ap, ps_pool, kk, col0, n_part=128):
                """PE-transpose src_ap [n_part, 128] -> dst3[:, kk, col0:col0+128]."""
                ptr = ps_pool.tile([128, 128], f32, tag="ptr")
                nc.tensor.transpose(ptr[:, 0:n_part], src_ap, ident[0:n_part, 0:n_part])
                nc.any.tensor_copy(out=dst3[:, kk, col0:col0 + 128], in_=ptr[:, 0:n_part])

            def ln_transpose(ps_pool):
                dst = htpool.tile([128, 2, T], f32r, tag="hT")
                for t in range(TT):
                    h = ln_block(x[:, t, :], ps_pool)
                    for kk in range(2):
                        transpose_into(dst, h[:, 128 * kk:128 * kk + 128], ps_pool, kk, 128 * t)
                return dst

            bcast_pool = None
            if with_bv or with_bproj or with_b2:
                bcast_pool = est.enter_context(tc.tile_pool(name="bcast", bufs=2))

            def bcast_add(dst_ap, bias_d_row):
                """dst[p, e] += bias[e] via a partition-broadcast DMA (general path)."""
                bb = bcast_pool.tile([128, E], f32, tag="bb")
                src = bass.AP(tensor=bias_d_row.tensor, offset=bias_d_row.offset,
                              ap=[[0, 128]] + list(bias_d_row.ap))
                nc.sync.dma_start(out=bb, in_=src)
                nc.vector.tensor_add(dst_ap, dst_ap, bb[:])

            # =================== trunk layers ===================
            for l in range(L):
                # -- layer weights --
                wq = wpool.tile([128, 2, E], f32r, tag="wq")
                nc.sync.dma_start(out=wq, in_=wq_d[l].rearrange("(k p) n -> p k n", p=128))
                wk = wpool.tile([128, 2, E], f32r, tag="wk")
                nc.sync.dma_start(out=wk, in_=wk_d[l].rearrange("(k p) n -> p k n", p=128))
                wv = wpool.tile([128, 2, E], f32r, tag="wv")
                nc.sync.dma_start(out=wv, in_=wv_d[l].rearrange("(k p) n -> p k n", p=128))
                wp = wpool.tile([128, 2, E], f32r, tag="wp")
                nc.sync.dma_start(out=wp, in_=wp_d[l].rearrange("(k p) n -> p k n", p=128))
                w1 = wpool.tile([128, 2, FF], f32r, tag="w1")
                nc.sync.dma_start(out=w1, in_=w1_d[l].rearrange("(k p) n -> p k n", p=128))
                w2 = wpool.tile([128, 8, E], f32r, tag="w2")
                nc.sync.dma_start(out=w2, in_=w2_d[l].rearrange("(k p) n -> p k n", p=128))
                bqs = wpool.tile([128, 2], f32, tag="bqs")
                nc.sync.dma_start(out=bqs, in_=bq_d[l].rearrange("(m p) -> p m", p=128))
                bks = wpool.tile([128, 2], f32, tag="bks")
                nc.sync.dma_start(out=bks, in_=bk_d[l].rearrange("(m p) -> p m", p=128))
                b1s = wpool.tile([128, 8], f32, tag="b1s")
                nc.sync.dma_start(out=b1s, in_=b1_d[l].rearrange("(m p) -> p m", p=128))

                # -- phase A: LN1, hT, qT/kT/v --
                with tc.tile_pool(name=f"psA{l}", bufs=2, space="PSUM") as psA:
                    hT = ln_transpose(psA)
                    qT = qkpool.tile([128, 2, T], f32r, tag="qT")
                    kT = qkpool.tile([128, 2, T], f32r, tag="kT")
                    for dst, w, bias in ((qT, wq, bqs), (kT, wk, bks)):
                        for m in range(2):
                            for j in range(2):
                                pq = psA.tile([128, 512], f32, tag="pqk")
                                for kk in range(2):
                                    nc.tensor.matmul(pq[:], w[:, kk, 128 * m:128 * m + 128],
                                                     hT[:, kk, 512 * j:512 * j + 512],
                                                     start=(kk == 0), stop=(kk == 1))
                                nc.scalar.activation(out=dst[:, m, 512 * j:512 * j + 512],
                                                     in_=pq[:], func=AF.Identity,
                                                     bias=bias[:, m:m + 1], scale=1.0)
                    for t in range(TT):
                        pv = psA.tile([128, E], f32, tag="pv")
                        for kk in range(2):
                            nc.tensor.matmul(pv[:], hT[:, kk, 128 * t:128 * t + 128],
                                             wv[:, kk, :], start=(kk == 0), stop=(kk == 1))
                        nc.any.tensor_copy(out=v_aug[:, t, :, 0:HD],
                                           in_=pv[:].rearrange("p (h d) -> p h d", h=H))
                        if with_bv:
                            bcast_add(v_aug[:, t, :, 0:HD].rearrange("p h d -> p (h d)"), bv_d[l])

                if debug and l == 0:
                    nc.sync.dma_start(out=hT0_d.rearrange("k p n -> p k n"), in_=hT[:].bitcast(f32))
                    nc.sync.dma_start(out=qT0_d.rearrange("k p n -> p k n"), in_=qT[:].bitcast(f32))
                    nc.sync.dma_start(out=v0_d, in_=v_aug[:].bitcast(f32))

                # -- phase B: attention (heads paired 2-at-a-time) --
                o_all = oapool.tile([128, TT, E], f32, tag="oall")
                with tc.tile_pool(name=f"psB{l}", bufs=2, space="PSUM") as psB, \
                     tc.tile_pool(name=f"psBo{l}", bufs=1, space="PSUM") as psBo, \
                     tc.tile_pool(name=f"psBt{l}", bufs=2, space="PSUM") as psBt:
                    for j in range(2):
                        i_hi = min(TT, 4 * j + 4)   # causal: valid tk tiles are i < i_hi
                        for pair in range(2):
                            po = psBo.tile([HD + 1, 2, 512], f32, tag="po")
                            for i in range(i_hi):
                                d = 128 * i - 512 * j
                                dlo = max(d, 0)
                                st = psB.tile([128, 2, 512], f32, tag="st")
                                for hh in range(2):
                                    hd = 2 * pair + hh
                                    r0 = HD * (hd % 2)
                                    nc.tensor.matmul(st[:, hh, dlo:512],
                                                     kT[r0:r0 + HD, hd // 2, 128 * i:128 * i + 128],
                                                     qT[r0:r0 + HD, hd // 2, 512 * j + dlo:512 * j + 512],
                                                     start=True, stop=True)
                                ex = expool.tile([128, 2, 512], f32r, tag="ex")
                                if dlo > 0:
                                    nc.vector.tensor_copy(out=ex[:, :, 0:dlo], in_=zeros2[:, :, 0:dlo])
                                nc.scalar.activation(out=ex[:, :, dlo:512], in_=st[:, :, dlo:512],
                                                     func=AF.Exp, scale=float(1.0 / np.sqrt(HD)))
                                if 0 <= d <= 384:
                                    nc.vector.tensor_mul(ex[:, :, d:d + 128], ex[:, :, d:d + 128], tri2[:])
                                for hh in range(2):
                                    nc.tensor.matmul(po[:, hh, :], v_aug[:, i, 2 * pair + hh, :],
                                                     ex[:, hh, :], start=(i == 0), stop=(i == i_hi - 1))
                            oa = aopool.tile([HD + 1, 2, 512], f32, tag="oa")
                            nc.any.tensor_copy(out=oa[:], in_=po[:])
                            for hh in range(2):
                                hd = 2 * pair + hh
                                for s in range(4):
                                    t = 4 * j + s
                                    ptr = psBt.tile([128, 128], f32, tag="ptr")
                                    nc.tensor.transpose(ptr[:, 0:HD + 1],
                                                        oa[:, hh, 128 * s:128 * s + 128],
                                                        ident[0:HD + 1, 0:HD + 1])
                                    r = rpool.tile([128, 1], f32, tag="r")
                                    nc.vector.reciprocal(out=r[:], in_=ptr[:, HD:HD + 1])
                                    nc.vector.tensor_scalar_mul(
                                        o_all[:, t, HD * hd:HD * hd + HD], ptr[:, 0:HD], r[:])

                if debug and l == 0:
                    nc.sync.dma_start(out=oall0_d, in_=o_all[:])

                # -- phase C: oT, proj, residual --
                with tc.tile_pool(name=f"psC{l}", bufs=2, space="PSUM") as psC:
                    oT = otpool.tile([128, 2, T], f32r, tag="oT")
                    for t in range(TT):
                        for kk in range(2):
                            transpose_into(oT, o_all[:, t, 128 * kk:128 * kk + 128], psC, kk, 128 * t)
                    for t in range(TT):
                        pp = psC.tile([128, E], f32, tag="pp")
                        for kk in range(2):
                            nc.tensor.matmul(pp[:], oT[:, kk, 128 * t:128 * t + 128],
                                             wp[:, kk, :], start=(kk == 0), stop=(kk == 1))
                        nc.vector.tensor_add(x[:, t, :], x[:, t, :], pp[:])
                        if with_bproj:
                            bcast_add(x[:, t, :], bpj_d[l])

                if debug:
                    nc.sync.dma_start(out=xmid_d[l].rearrange("(t p) e -> p t e", p=128), in_=x[:])

                # -- phase D: LN2, FFN, residual --
                with tc.tile_pool(name=f"psD{l}", bufs=2, space="PSUM") as psD, \
                     tc.tile_pool(name=f"psD2{l}", bufs=2, space="PSUM") as psD2:
                    h2T = ln_transpose(psD)
                    ff1T = f1pool.tile([128, 8, T], f32r, tag="f1T")
                    for fm in range(8):
                        for j in range(2):
                            pf = psD.tile([128, 512], f32, tag="pf")
                            for kk in range(2):
                                nc.tensor.matmul(pf[:], w1[:, kk, 128 * fm:128 * fm + 128],
                                                 h2T[:, kk, 512 * j:512 * j + 512],
                                                 start=(kk == 0), stop=(kk == 1))
                            nc.scalar.activation(out=ff1T[:, fm, 512 * j:512 * j + 512], in_=pf[:],
                                                 func=AF.Relu, bias=b1s[:, fm:fm + 1], scale=1.0)
                    for t in range(TT):
                        pf2 = psD2.tile([128, E], f32, tag="pf2")
                        for fm in range(8):
                            nc.tensor.matmul(pf2[:], ff1T[:, fm, 128 * t:128 * t + 128],
                                             w2[:, fm, :], start=(fm == 0), stop=(fm == 7))
                        nc.vector.tensor_add(x[:, t, :], x[:, t, :], pf2[:])
                        if with_b2:
                            bcast_add(x[:, t, :], b2_d[l])
                if debug:
                    nc.sync.dma_start(out=xdbg_d[l].rearrange("(t p) e -> p t e", p=128), in_=x[:])

            # =================== vocab head ===================
            rs = rspool.tile([128, TT, NVC], f32)
            with tc.tile_pool(name="psF", bufs=2, space="PSUM") as psF, \
                 tc.tile_pool(name="psL", bufs=5, space="PSUM") as psL:
                xT = ln_transpose(psF)
                wo_r = wo_d.rearrange("(k p) n -> p k n", p=128)
                for vc in range(NVC):
                    n = min(VCH, VH - VCH * vc)
                    wo = wopool.tile([128, 2, VCH], f32r, tag="wo")
                    nc.sync.dma_start(out=wo[:, :, 0:n], in_=wo_r[:, :, VCH * vc:VCH * vc + n])
                    for t in range(TT):
                        pl = psL.tile([128, VCH], f32, tag="pl")
                        nc.tensor.matmul(pl[:, 0:n], xT[:, 0, 128 * t:128 * t + 128],
                                         wo[:, 0, 0:n], start=True, stop=False)
                        nc.tensor.matmul(pl[:, 0:n], xT[:, 1, 128 * t:128 * t + 128],
                                         wo[:, 1, 0:n], start=False, stop=not with_bout)
                        if with_bout:
                            nc.tensor.matmul(pl[:, 0:n], ones_row[0:1, :],
                                             bo_sb[0:1, VCH * vc:VCH * vc + n],
                                             start=False, stop=True)
                        ed = edpool.tile([128, VCH], bf16, tag="ed")
                        nc.scalar.activation(out=ed[:, 0:n], in_=pl[:, 0:n], func=AF.Exp,
                                             scale=1.0, accum_out=rs[:, t, vc:vc + 1])
                        lst = lstpool.tile([128, VCH], f16, tag="lst")
                        nc.vector.tensor_copy(out=lst[:, 0:n], in_=pl[:, 0:n])
                        nc.sync.dma_start(out=logits_d[128 * t:128 * t + 128, VCH * vc:VCH * vc + n],
                                          in_=lst[:, 0:n])
                rssum = const.tile([128, TT], f32)
                for t in range(TT):
                    nc.vector.reduce_sum(out=rssum[:, t:t + 1], in_=rs[:, t, :],
                                         axis=mybir.AxisListType.X)
                nc.sync.dma_start(out=rowsum_d.rearrange("(t p) -> p t", p=128), in_=rssum[:])

    nc.compile()
    return nc


def _get_program(key):
    if key not in _cache:
        _cache[key] = _build_program(*key)
    return _cache[key]


def kernel(idx, targets, tok_emb, pos_emb, Wq, Wk, Wv, Wproj, bproj,
           W1, b1, W2, b2, ln1_g, ln1_b, ln2_g, ln2_b, lnf_g, lnf_b,
           Wout, bout):
    from concourse.bass_utils import run_bass_kernel_spmd

    idx = np.asarray(idx)
    targets = np.asarray(targets)
    f32 = np.float32
    tok_emb = np.asarray(tok_emb, f32)
    pos_emb = np.asarray(pos_emb, f32)

    # ---- host-side weight prep: fold LN gains/biases into adjacent matmuls ----
    ln1_g, ln1_b = np.asarray(ln1_g, f32), np.asarray(ln1_b, f32)
    ln2_g, ln2_b = np.asarray(ln2_g, f32), np.asarray(ln2_b, f32)
    lnf_g, lnf_b = np.asarray(lnf_g, f32), np.asarray(lnf_b, f32)
    Wq, Wk, Wv = np.asarray(Wq, f32), np.asarray(Wk, f32), np.asarray(Wv, f32)
    Wproj, W1, W2 = np.asarray(Wproj, f32), np.asarray(W1, f32), np.asarray(W2, f32)
    Wout = np.asarray(Wout, f32)
    b1 = np.asarray(b1, f32)
    bproj, b2, bout = np.asarray(bproj, f32), np.asarray(b2, f32), np.asarray(bout, f32)

    g1 = ln1_g[:, :, None]
    wq_e = np.ascontiguousarray(g1 * Wq)
    wk_e = np.ascontiguousarray(g1 * Wk)
    wv_e = np.ascontiguousarray(g1 * Wv)
    bq_e = np.einsum("le,leo->lo", ln1_b, Wq).astype(f32)
    bk_e = np.einsum("le,leo->lo", ln1_b, Wk).astype(f32)
    bv_e = np.einsum("le,leo->lo", ln1_b, Wv).astype(f32)
    w1_e = np.ascontiguousarray(ln2_g[:, :, None] * W1)
    b1_e = (b1 + np.einsum("le,leo->lo", ln2_b, W1)).astype(f32)
    wo_e = np.ascontiguousarray(lnf_g[:, None] * Wout)
    bo_e = (bout + lnf_b @ Wout).astype(f32)

    with_bv = bool(np.any(bv_e))
    with_bproj = bool(np.any(bproj))
    with_b2 = bool(np.any(b2))
    with_bout = bool(np.any(bo_e))

    # ---- embedding gather (host) ----
    x0 = tok_emb[idx.astype(np.int64)] + pos_emb[None, :, :]   # [B, T, E] f32

    # ---- constants ----
    tri = np.triu(np.ones((128, 128), f32))          # keep tk_local <= tq_local
    tri2 = np.ascontiguousarray(np.broadcast_to(tri[:, None, :], (128, 2, 128)))
    ident = np.eye(128, dtype=f32)
    zeros2 = np.zeros((128, 2, VCH), f32)
    ones_aug = np.ones((128, TT * H), f32)
    ones_row = np.ones((1, 128), f32)

    nc = _get_program((with_bv, with_bproj, with_b2, with_bout))

    in_maps = []
    for c in range(NCORES):
        b, par = c // 2, c % 2
        m = {
            "x0": np.ascontiguousarray(x0[b]),
            "wq": wq_e, "wk": wk_e, "wv": wv_e, "wp": Wproj,
            "w1": w1_e, "w2": W2,
            "bq": bq_e, "bk": bk_e, "b1": b1_e,
            "wout": np.ascontiguousarray(wo_e[:, par * VH:(par + 1) * VH]),
            "tri2": tri2, "ident": ident, "zeros2": zeros2, "ones_aug": ones_aug,
        }
        if with_bv:
            m["bvx"] = bv_e
        if with_bproj:
            m["bpj"] = bproj
        if with_b2:
            m["b2x"] = b2
        if with_bout:
            m["bout"] = np.ascontiguousarray(bo_e[par * VH:(par + 1) * VH])
            m["ones_row"] = ones_row
        in_maps.append(m)

    res = run_bass_kernel_spmd(nc, in_maps, core_ids=list(range(NCORES)))

    # ---- host-side assembly ----
    logits = np.empty((B * T, V), f32)
    rowsum = np.zeros((B, T), np.float64)
    for c in range(NCORES):
        b, par = c // 2, c % 2
        logits[b * T:(b + 1) * T, par * VH:(par + 1) * VH] = \
            res.results[c]["logits_h"].astype(f32)
        rowsum[b] += res.results[c]["rowsum"].astype(np.float64)
    lse = np.log(rowsum).reshape(-1)
    tgt = targets.reshape(-1).astype(np.int64)
    tl = logits[np.arange(B * T), tgt].astype(np.float64)
    loss = np.array(-(tl - lse).mean(), dtype=f32)
    return logits, loss


# revision 8
# speedup vs baseline: 1.2068x; 1.2068x over previous
# Trainium2 Bass kernel for nn_BigramLanguageModel (4-layer dense transformer + vocab head).
#
# Sharding (8 NeuronCores, no collectives):
#   - Trunk (4 transformer layers): data-parallel over batch. Core c computes the
#     full trunk for batch element c//2 over all T=1024 tokens (each batch is
#     computed by the 2 cores of a pair; redundant but exchange-free).
#   - Vocab head: tensor-parallel over V. Core c projects all 1024 tokens of its
#     batch against its half of Wout (V/2 = 16000 columns, parity c%2), writes
#     float16 logits and fused exp-row-sum partials for the loss.
#   - Host: embedding gather, LN gain/bias folding into weights, logits assembly
#     (f16 -> f32), log-sum-exp combine across vocab halves, final loss.
#
# Trunk matmuls run in float32r (full PE speed, ~1.5e-4 rel error); the vocab
# projection runs in bf16 (fast weight load) which dominates the remaining error.
import numpy as np

B, T, E, H, L, V = 4, 1024, 256, 4, 4, 32000
HD = E // H              # 64
EPS = 1e-5
NCORES = 8
VH = V // 2              # 16000 vocab columns per core
FF = 4 * E               # 1024
TT = T // 128            # 8 token tiles
GV = 2048                # vocab group (4 PSUM banks)
NG = (VH + GV - 1) // GV  # 8 groups (7x2048 + 1x1664)

_cache = {}


def _build_program(with_bqk, with_bv, with_bproj, with_b2, with_bout, debug=False):
    import concourse.bass as bass
    import concourse.tile as tile
    from concourse import bacc, mybir

    f32 = mybir.dt.float32
    f32r = mybir.dt.float32r
    f16 = mybir.dt.float16
    bf16 = mybir.dt.bfloat16
    AF = mybir.ActivationFunctionType
    MUL = mybir.AluOpType.mult
    ADD = mybir.AluOpType.add

    nc = bacc.Bacc("TRN2", target_bir_lowering=False, debug=False, num_devices=NCORES)

    # ---- DRAM I/O ----
    x0_d = nc.dram_tensor("x0", [T, E], f32, kind="ExternalInput").ap()
    wq_d = nc.dram_tensor("wq", [L, E, E], f32r, kind="ExternalInput").ap()
    wk_d = nc.dram_tensor("wk", [L, E, E], f32r, kind="ExternalInput").ap()
    wv_d = nc.dram_tensor("wv", [L, E, E], f32r, kind="ExternalInput").ap()
    wp_d = nc.dram_tensor("wp", [L, E, E], f32r, kind="ExternalInput").ap()
    w1_d = nc.dram_tensor("w1", [L, E, FF], f32r, kind="ExternalInput").ap()
    w2_d = nc.dram_tensor("w2", [L, FF, E], f32r, kind="ExternalInput").ap()
    b1_d = nc.dram_tensor("b1", [L, FF], f32, kind="ExternalInput").ap()
    bq_d = nc.dram_tensor("bq", [L, E], f32, kind="ExternalInput").ap() if with_bqk else None
    bk_d = nc.dram_tensor("bk", [L, E], f32, kind="ExternalInput").ap() if with_bqk else None
    bv_d = nc.dram_tensor("bvx", [L, E], f32, kind="ExternalInput").ap() if with_bv else None
    bpj_d = nc.dram_tensor("bpj", [L, E], f32, kind="ExternalInput").ap() if with_bproj else None
    b2_d = nc.dram_tensor("b2x", [L, E], f32, kind="ExternalInput").ap() if with_b2 else None
    wo_d = nc.dram_tensor("wout", [E, VH], bf16, kind="ExternalInput").ap()
    bo_d = nc.dram_tensor("bout", [VH], bf16, kind="ExternalInput").ap() if with_bout else None
    tri_d = nc.dram_tensor("tri2", [128, 2, 128], f32r, kind="ExternalInput").ap()
    id_d = nc.dram_tensor("ident", [128, 128], f32, kind="ExternalInput").ap()
    z2_d = nc.dram_tensor("zeros2", [128, 2, 512], f32r, kind="ExternalInput").ap()
    oa_d = nc.dram_tensor("ones_aug", [128, TT * H], f32r, kind="ExternalInput").ap()
    on_d = nc.dram_tensor("ones_row", [1, 128], bf16, kind="ExternalInput").ap() if with_bout else None
    logits_d = nc.dram_tensor("logits_h", [T, VH], f16, kind="ExternalOutput").ap()
    rowsum_d = nc.dram_tensor("rowsum", [T], f32, kind="ExternalOutput").ap()
    if debug:
        xdbg_d = nc.dram_tensor("xdbg", [L, T, E], f32, kind="ExternalOutput").ap()

    with tile.TileContext(nc) as tc:
        from contextlib import ExitStack
        est = ExitStack()
        with est:
            # ---- kernel-lifetime SBUF pools ----
            const = est.enter_context(tc.tile_pool(name="const", bufs=1))
            xpool = est.enter_context(tc.tile_pool(name="x", bufs=1))
            vapool = est.enter_context(tc.tile_pool(name="vaug", bufs=1))
            htpool = est.enter_context(tc.tile_pool(name="ht", bufs=2))
            qkpool = est.enter_context(tc.tile_pool(name="qk", bufs=1))
            otpool = est.enter_context(tc.tile_pool(name="ot", bufs=1))
            oapool = est.enter_context(tc.tile_pool(name="oall", bufs=1))
            wpool = est.enter_context(tc.tile_pool(name="w", bufs=2))
            hpool = est.enter_context(tc.tile_pool(name="h", bufs=3))
            spool = est.enter_context(tc.tile_pool(name="stats", bufs=3))
            expool = est.enter_context(tc.tile_pool(name="ex", bufs=3))
            aopool = est.enter_context(tc.tile_pool(name="oaT", bufs=2))
            f1pool = est.enter_context(tc.tile_pool(name="ff1", bufs=1))
            rpool = est.enter_context(tc.tile_pool(name="recip", bufs=8))
            wopool = est.enter_context(tc.tile_pool(name="wo", bufs=2))
            lstpool = est.enter_context(tc.tile_pool(name="lst", bufs=2))
            edpool = est.enter_context(tc.tile_pool(name="ed", bufs=1))
            rspool = est.enter_context(tc.tile_pool(name="rs", bufs=1))

            # ---- constants ----
            tri2 = const.tile([128, 2, 128], f32r)
            nc.sync.dma_start(out=tri2, in_=tri_d)
            ident = const.tile([128, 128], f32)
            nc.sync.dma_start(out=ident, in_=id_d)
            zeros2 = const.tile([128, 2, 512], f32r)
            nc.sync.dma_start(out=zeros2, in_=z2_d)
            eps_sb = const.tile([128, 1], f32)
            nc.vector.memset(eps_sb[:], EPS)
            if with_bout:
                ones_row = const.tile([1, 128], bf16)
                nc.sync.dma_start(out=ones_row, in_=on_d)
                bo_sb = const.tile([1, VH], bf16)
                nc.sync.dma_start(out=bo_sb, in_=bo_d[None, :])

            # ---- residual stream + v_aug ----
            x = xpool.tile([128, TT, E], f32)
            nc.sync.dma_start(out=x, in_=x0_d.rearrange("(t p) e -> p t e", p=128))
            v_aug = vapool.tile([128, TT, H, HD + 1], f32r)
            nc.sync.dma_start(out=v_aug[:, :, :, HD], in_=oa_d.rearrange("p (t h) -> p t h", t=TT))

            def ln_pass(ps_pool, out_dtype):
                """LayerNorm standardize of all TT tiles of x + transpose into a
                feature-major [128, 2, T] tile of out_dtype. Gains/biases are folded
                into downstream weights on the host."""
                mv8 = spool.tile([128, TT, 2], f32, tag="mv8")
                for t in range(TT):
                    st6 = spool.tile([128, 6], f32, tag="st6")
                    nc.vector.bn_stats(out=st6[:], in_=x[:, t, :])
                    nc.vector.bn_aggr(out=mv8[:, t, :], in_=st6[:])
                sd8 = spool.tile([128, TT], f32, tag="sd8")
                nc.scalar.activation(out=sd8[:], in_=mv8[:, :, 1], func=AF.Sqrt,
                                     bias=eps_sb[:, 0:1], scale=1.0)
                rstd8 = spool.tile([128, TT], f32, tag="rstd8")
                nc.vector.reciprocal(out=rstd8[:], in_=sd8[:])
                nmr8 = spool.tile([128, TT], f32, tag="nmr8")
                nc.vector.tensor_mul(nmr8[:], mv8[:, :, 0], rstd8[:])
                nc.vector.tensor_scalar_mul(nmr8[:], nmr8[:], -1.0)
                dst = htpool.tile([128, 2, T], out_dtype, tag="hT")
                for t in range(TT):
                    h = hpool.tile([128, E], f32, tag="h")
                    nc.vector.tensor_scalar(out=h[:], in0=x[:, t, :],
                                            scalar1=rstd8[:, t:t + 1],
                                            scalar2=nmr8[:, t:t + 1],
                                            op0=MUL, op1=ADD)
                    for kk in range(2):
                        ptr = ps_pool.tile([128, 128], f32, tag="ptr")
                        nc.tensor.transpose(ptr[:], h[:, 128 * kk:128 * kk + 128], ident[:])
                        nc.scalar.copy(out=dst[:, kk, 128 * t:128 * t + 128], in_=ptr[:])
                return dst

            bcast_pool = None
            if with_bv or with_bproj or with_b2:
                bcast_pool = est.enter_context(tc.tile_pool(name="bcast", bufs=2))

            def bcast_add(dst_ap, bias_d_row):
                bb = bcast_pool.tile([128, E], f32, tag="bb")
                src = bass.AP(tensor=bias_d_row.tensor, offset=bias_d_row.offset,
                              ap=[[0, 128]] + list(bias_d_row.ap))
                nc.sync.dma_start(out=bb, in_=src)
                nc.vector.tensor_add(dst_ap, dst_ap, bb[:])

            # =================== trunk layers ===================
            for l in range(L):
                wq = wpool.tile([128, 2, E], f32r, tag="wq")
                nc.sync.dma_start(out=wq, in_=wq_d[l].rearrange("(k p) n -> p k n", p=128))
                wk = wpool.tile([128, 2, E], f32r, tag="wk")
                nc.sync.dma_start(out=wk, in_=wk_d[l].rearrange("(k p) n -> p k n", p=128))
                wv = wpool.tile([128, 2, E], f32r, tag="wv")
                nc.sync.dma_start(out=wv, in_=wv_d[l].rearrange("(k p) n -> p k n", p=128))
                wp = wpool.tile([128, 2, E], f32r, tag="wp")
                nc.sync.dma_start(out=wp, in_=wp_d[l].rearrange("(k p) n -> p k n", p=128))
                w1 = wpool.tile([128, 2, FF], f32r, tag="w1")
                nc.sync.dma_start(out=w1, in_=w1_d[l].rearrange("(k p) n -> p k n", p=128))
                w2 = wpool.tile([128, 8, E], f32r, tag="w2")
                nc.sync.dma_start(out=w2, in_=w2_d[l].rearrange("(k p) n -> p k n", p=128))
                b1s = wpool.tile([128, 8], f32, tag="b1s")
                nc.sync.dma_start(out=b1s, in_=b1_d[l].rearrange("(m p) -> p m", p=128))
                if with_bqk:
                    bqs = wpool.tile([128, 2], f32, tag="bqs")
                    nc.sync.dma_start(out=bqs, in_=bq_d[l].rearrange("(m p) -> p m", p=128))
                    bks = wpool.tile([128, 2], f32, tag="bks")
                    nc.sync.dma_start(out=bks, in_=bk_d[l].rearrange("(m p) -> p m", p=128))

                # -- phase A: LN1, hT, qT/kT/v --
                with tc.tile_pool(name=f"psA{l}", bufs=2, space="PSUM") as psA:
                    hT = ln_pass(psA, f32r)
                    qT = qkpool.tile([128, 2, T], f32r, tag="qT")
                    kT = qkpool.tile([128, 2, T], f32r, tag="kT")
                    for di, (dst, w) in enumerate(((qT, wq), (kT, wk))):
                        for m in range(2):
                            for j in range(2):
                                pq = psA.tile([128, 512], f32, tag="pqk")
                                for kk in range(2):
                                    nc.tensor.matmul(pq[:], w[:, kk, 128 * m:128 * m + 128],
                                                     hT[:, kk, 512 * j:512 * j + 512],
                                                     start=(kk == 0), stop=(kk == 1))
                                if with_bqk:
                                    bias = (bqs if di == 0 else bks)[:, m:m + 1]
                                    nc.scalar.activation(out=dst[:, m, 512 * j:512 * j + 512],
                                                         in_=pq[:], func=AF.Identity,
                                                         bias=bias, scale=1.0)
                                else:
                                    nc.vector.tensor_copy(out=dst[:, m, 512 * j:512 * j + 512],
                                                          in_=pq[:])
                    for t in range(TT):
                        pv = psA.tile([128, E], f32, tag="pv")
                        for kk in range(2):
                            nc.tensor.matmul(pv[:], hT[:, kk, 128 * t:128 * t + 128],
                                             wv[:, kk, :], start=(kk == 0), stop=(kk == 1))
                        nc.vector.tensor_copy(out=v_aug[:, t, :, 0:HD],
                                              in_=pv[:].rearrange("p (h d) -> p h d", h=H))
                        if with_bv:
                            bcast_add(v_aug[:, t, :, 0:HD].rearrange("p h d -> p (h d)"), bv_d[l])

                # -- phase B: attention (heads paired 2-at-a-time) --
                o_all = oapool.tile([128, TT, E], f32, tag="oall")
                with tc.tile_pool(name=f"psB{l}", bufs=2, space="PSUM") as psB, \
                     tc.tile_pool(name=f"psBo{l}", bufs=1, space="PSUM") as psBo, \
                     tc.tile_pool(name=f"psBt{l}", bufs=2, space="PSUM") as psBt:
                    for j in range(2):
                        i_hi = min(TT, 4 * j + 4)   # causal: valid tk tiles are i < i_hi
                        for pair in range(2):
                            po = psBo.tile([HD + 1, 2, 512], f32, tag="po")
                            for i in range(i_hi):
                                d = 128 * i - 512 * j
                                dlo = max(d, 0)
                                st = psB.tile([128, 2, 512], f32, tag="st")
                                for hh in range(2):
                                    hd = 2 * pair + hh
                                    r0 = HD * (hd % 2)
                                    nc.tensor.matmul(st[:, hh, dlo:512],
                                                     kT[r0:r0 + HD, hd // 2, 128 * i:128 * i + 128],
                                                     qT[r0:r0 + HD, hd // 2, 512 * j + dlo:512 * j + 512],
                                                     start=True, stop=True)
                                ex = expool.tile([128, 2, 512], f32r, tag="ex")
                                if dlo > 0:
                                    nc.vector.tensor_copy(out=ex[:, :, 0:dlo], in_=zeros2[:, :, 0:dlo])
                                nc.scalar.activation(out=ex[:, :, dlo:512], in_=st[:, :, dlo:512],
                                                     func=AF.Exp, scale=float(1.0 / np.sqrt(HD)))
                                if 0 <= d <= 384:
                                    nc.vector.tensor_mul(ex[:, :, d:d + 128], ex[:, :, d:d + 128], tri2[:])
                                for hh in range(2):
                                    nc.tensor.matmul(po[:, hh, :], v_aug[:, i, 2 * pair + hh, :],
                                                     ex[:, hh, :], start=(i == 0), stop=(i == i_hi - 1))
                            oa = aopool.tile([HD + 1, 2, 512], f32, tag="oa")
                            nc.scalar.copy(out=oa[:], in_=po[:])
                            for hh in range(2):
                                hd = 2 * pair + hh
                                for s in range(4):
                                    t = 4 * j + s
                                    ptr = psBt.tile([128, 128], f32, tag="ptr")
                                    nc.tensor.transpose(ptr[:, 0:HD + 1],
                                                        oa[:, hh, 128 * s:128 * s + 128],
                                                        ident[0:HD + 1, 0:HD + 1])
                                    r = rpool.tile([128, 1], f32, tag="r")
                                    nc.vector.reciprocal(out=r[:], in_=ptr[:, HD:HD + 1])
                                    nc.vector.tensor_scalar_mul(
                                        o_all[:, t, HD * hd:HD * hd + HD], ptr[:, 0:HD], r[:])

                # -- phase C: oT, proj, residual --
                with tc.tile_pool(name=f"psC{l}", bufs=2, space="PSUM") as psC:
                    oT = otpool.tile([128, 2, T], f32r, tag="oT")
                    for t in range(TT):
                        for kk in range(2):
                            ptr = psC.tile([128, 128], f32, tag="ptr")
                            nc.tensor.transpose(ptr[:], o_all[:, t, 128 * kk:128 * kk + 128], ident[:])
                            nc.vector.tensor_copy(out=oT[:, kk, 128 * t:128 * t + 128], in_=ptr[:])
                    for t in range(TT):
                        pp = psC.tile([128, E], f32, tag="pp")
                        for kk in range(2):
                            nc.tensor.matmul(pp[:], oT[:, kk, 128 * t:128 * t + 128],
                                             wp[:, kk, :], start=(kk == 0), stop=(kk == 1))
                        nc.vector.tensor_add(x[:, t, :], x[:, t, :], pp[:])
                        if with_bproj:
                            bcast_add(x[:, t, :], bpj_d[l])

                # -- phase D: LN2, FFN, residual --
                with tc.tile_pool(name=f"psD{l}", bufs=2, space="PSUM") as psD, \
                     tc.tile_pool(name=f"psD2{l}", bufs=2, space="PSUM") as psD2:
                    h2T = ln_pass(psD, f32r)
                    ff1T = f1pool.tile([128, 8, T], f32r, tag="f1T")
                    for fm in range(8):
                        for j in range(2):
                            pf = psD.tile([128, 512], f32, tag="pf")
                            for kk in range(2):
                                nc.tensor.matmul(pf[:], w1[:, kk, 128 * fm:128 * fm + 128],
                                                 h2T[:, kk, 512 * j:512 * j + 512],
                                                 start=(kk == 0), stop=(kk == 1))
                            nc.scalar.activation(out=ff1T[:, fm, 512 * j:512 * j + 512], in_=pf[:],
                                                 func=AF.Relu, bias=b1s[:, fm:fm + 1], scale=1.0)
                    for t in range(TT):
                        pf2 = psD2.tile([128, E], f32, tag="pf2")
                        for fm in range(8):
                            nc.tensor.matmul(pf2[:], ff1T[:, fm, 128 * t:128 * t + 128],
                                             w2[:, fm, :], start=(fm == 0), stop=(fm == 7))
                        nc.vector.tensor_add(x[:, t, :], x[:, t, :], pf2[:])
                        if with_b2:
                            bcast_add(x[:, t, :], b2_d[l])
                if debug:
                    nc.sync.dma_start(out=xdbg_d[l].rearrange("(t p) e -> p t e", p=128), in_=x[:])

            # =================== vocab head (bf16) ===================
            rs = rspool.tile([128, TT, NG], f32)
            with tc.tile_pool(name="psF", bufs=2, space="PSUM") as psF:
                xT = ln_pass(psF, bf16)
            wo_r = wo_d.rearrange("(k p) n -> p k n", p=128)
            with tc.tile_pool(name="psL", bufs=2, space="PSUM") as psL:
                for g in range(NG):
                    n = min(GV, VH - GV * g)
                    wo = wopool.tile([128, 2, GV], bf16, tag="wo")
                    nc.sync.dma_start(out=wo[:, :, 0:n], in_=wo_r[:, :, GV * g:GV * g + n])
                    for t in range(TT):
                        pl = psL.tile([128, GV], f32, tag="pl")
                        for c4 in range(4):
                            w = min(512, n - 512 * c4)
                            if w <= 0:
                                break
                            nc.tensor.matmul(pl[:, 512 * c4:512 * c4 + w],
                                             xT[:, 0, 128 * t:128 * t + 128],
                                             wo[:, 0, 512 * c4:512 * c4 + w],
                                             start=True, stop=False)
                            nc.tensor.matmul(pl[:, 512 * c4:512 * c4 + w],
                                             xT[:, 1, 128 * t:128 * t + 128],
                                             wo[:, 1, 512 * c4:512 * c4 + w],
                                             start=False, stop=not with_bout)
                            if with_bout:
                                nc.tensor.matmul(pl[:, 512 * c4:512 * c4 + w],
                                                 ones_row[0:1, :],
                                                 bo_sb[0:1, GV * g + 512 * c4:GV * g + 512 * c4 + w],
                                                 start=False, stop=True)
                        ed = edpool.tile([128, GV], bf16, tag="ed")
                        nc.scalar.activation(out=ed[:, 0:n], in_=pl[:, 0:n], func=AF.Exp,
                                             scale=1.0, accum_out=rs[:, t, g:g + 1])
                        lst = lstpool.tile([128, GV], f16, tag="lst")
                        nc.vector.tensor_copy(out=lst[:, 0:n], in_=pl[:, 0:n])
                        nc.sync.dma_start(out=logits_d[128 * t:128 * t + 128, GV * g:GV * g + n],
                                          in_=lst[:, 0:n])
                rssum = const.tile([128, TT], f32)
                for t in range(TT):
                    nc.vector.reduce_sum(out=rssum[:, t:t + 1], in_=rs[:, t, :],
                                         axis=mybir.AxisListType.X)
                nc.sync.dma_start(out=rowsum_d.rearrange("(t p) -> p t", p=128), in_=rssum[:])

    nc.compile()
    return nc


def _get_program(key):
    if key not in _cache:
        _cache[key] = _build_program(*key)
    return _cache[key]


def kernel(idx, targets, tok_emb, pos_emb, Wq, Wk, Wv, Wproj, bproj,
           W1, b1, W2, b2, ln1_g, ln1_b, ln2_g, ln2_b, lnf_g, lnf_b,
           Wout, bout):
    import ml_dtypes
    from concourse.bass_utils import run_bass_kernel_spmd

    idx = np.asarray(idx)
    targets = np.asarray(targets)
    f32 = np.float32
    tok_emb = np.asarray(tok_emb, f32)
    pos_emb = np.asarray(pos_emb, f32)

    # ---- host-side weight prep: fold LN gains/biases into adjacent matmuls ----
    ln1_g, ln1_b = np.asarray(ln1_g, f32), np.asarray(ln1_b, f32)
    ln2_g, ln2_b = np.asarray(ln2_g, f32), np.asarray(ln2_b, f32)
    lnf_g, lnf_b = np.asarray(lnf_g, f32), np.asarray(lnf_b, f32)
    Wq, Wk, Wv = np.asarray(Wq, f32), np.asarray(Wk, f32), np.asarray(Wv, f32)
    Wproj, W1, W2 = np.asarray(Wproj, f32), np.asarray(W1, f32), np.asarray(W2, f32)
    Wout = np.asarray(Wout, f32)
    b1 = np.asarray(b1, f32)
    bproj, b2, bout = np.asarray(bproj, f32), np.asarray(b2, f32), np.asarray(bout, f32)

    g1 = ln1_g[:, :, None]
    wq_e = np.ascontiguousarray(g1 * Wq)
    wk_e = np.ascontiguousarray(g1 * Wk)
    wv_e = np.ascontiguousarray(g1 * Wv)
    bq_e = np.einsum("le,leo->lo", ln1_b, Wq).astype(f32)
    bk_e = np.einsum("le,leo->lo", ln1_b, Wk).astype(f32)
    bv_e = np.einsum("le,leo->lo", ln1_b, Wv).astype(f32)
    w1_e = np.ascontiguousarray(ln2_g[:, :, None] * W1)
    b1_e = (b1 + np.einsum("le,leo->lo", ln2_b, W1)).astype(f32)
    wo_e = np.ascontiguousarray((lnf_g[:, None] * Wout).astype(ml_dtypes.bfloat16))
    bo_e = (bout + lnf_b @ Wout).astype(f32)

    with_bqk = bool(np.any(bq_e)) or bool(np.any(bk_e))
    with_bv = bool(np.any(bv_e))
    with_bproj = bool(np.any(bproj))
    with_b2 = bool(np.any(b2))
    with_bout = bool(np.any(bo_e))

    # ---- embedding gather (host) ----
    x0 = tok_emb[idx.astype(np.int64)] + pos_emb[None, :, :]   # [B, T, E] f32

    # ---- constants ----
    tri = np.triu(np.ones((128, 128), f32))          # keep tk_local <= tq_local
    tri2 = np.ascontiguousarray(np.broadcast_to(tri[:, None, :], (128, 2, 128)))
    ident = np.eye(128, dtype=f32)
    zeros2 = np.zeros((128, 2, 512), f32)
    ones_aug = np.ones((128, TT * H), f32)
    ones_row = np.ones((1, 128), ml_dtypes.bfloat16)

    nc = _get_program((with_bqk, with_bv, with_bproj, with_b2, with_bout))

    in_maps = []
    for c in range(NCORES):
        b, par = c // 2, c % 2
        m = {
            "x0": np.ascontiguousarray(x0[b]),
            "wq": wq_e, "wk": wk_e, "wv": wv_e, "wp": Wproj,
            "w1": w1_e, "w2": W2, "b1": b1_e,
            "wout": np.ascontiguousarray(wo_e[:, par * VH:(par + 1) * VH]),
            "tri2": tri2, "ident": ident, "zeros2": zeros2, "ones_aug": ones_aug,
        }
        if with_bqk:
            m["bq"] = bq_e
            m["bk"] = bk_e
        if with_bv:
            m["bvx"] = bv_e
        if with_bproj:
            m["bpj"] = bproj
        if with_b2:
            m["b2x"] = b2
        if with_bout:
            m["bout"] = np.ascontiguousarray(bo_e[par * VH:(par + 1) * VH]).astype(ml_dtypes.bfloat16)
            m["ones_row"] = ones_row
        in_maps.append(m)

    res = run_bass_kernel_spmd(nc, in_maps, core_ids=list(range(NCORES)))

    # ---- host-side assembly ----
    logits = np.empty((B * T, V), f32)
    rowsum = np.zeros((B, T), np.float64)
    for c in range(NCORES):
        b, par = c // 2, c % 2
        logits[b * T:(b + 1) * T, par * VH:(par + 1) * VH] = \
            res.results[c]["logits_h"].astype(f32)
        rowsum[b] += res.results[c]["rowsum"].astype(np.float64)
    lse = np.log(rowsum).reshape(-1)
    tgt = targets.reshape(-1).astype(np.int64)
    tl = logits[np.arange(B * T), tgt].astype(np.float64)
    loss = np.array(-(tl - lse).mean(), dtype=f32)
    return logits, loss
